# revision 4
# baseline (speedup 1.0000x reference)
"""TopK sparse autoencoder (encode -> per-token top-100 mask -> decode) on 8 TRN2 cores.

Sharding: data-parallel over the 4096-token batch (512 tokens/core), weights
replicated. Per core:
  pre  = (x - b_dec) @ W_enc + b_enc          (fp32 matmul on PE, exact selection)
  t    = 100th largest of relu(pre) per token (DVE max8/match_replace:
         top-24 of each 512-wide chunk extracted destructively from PSUM
         during evacuation, then exact top-100 of the 768 candidates)
  E    = pre * (pre >= t)                     (masked in transposed layout)
  xhat = E @ W_dec + b_dec                    (bf16 matmul, E^T tiles stationary)

The top-24-per-chunk candidate set provably contains the global top-100 as
long as no 512-chunk holds more than 24 of a row's top-100 (true with huge
margin for iid inputs; max observed is 15).
"""
import numpy as np
import ml_dtypes

import concourse.bacc as bacc
import concourse.mybir as mybir
from concourse.tile import TileContext
from concourse.masks import make_identity
from concourse.bass_utils import run_bass_kernel_spmd

B, DIN, DSAE, TOPK = 4096, 2048, 16384, 100
NCORES = 8
TPC = B // NCORES            # 512 tokens per core
MT = TPC // 128              # 4 token tiles per core
CH = 512                     # encode chunk width == one PSUM bank (fp32)
NCH = DSAE // CH             # 32 chunks
KTE = DIN // 128             # 16 contraction slices for encode
KTD = DSAE // 128            # 128 contraction slices for decode
R_EXT = 3                    # extraction rounds per chunk -> top-24 candidates
NCAND = NCH * R_EXT * 8      # 768 candidates per token
NEG = -1e30

_cache = {}


def _build(with_benc: bool, with_bdec: bool):
    key = (with_benc, with_bdec)
    if key in _cache:
        return _cache[key]

    nc = bacc.Bacc()
    x_d = nc.dram_tensor("x", [TPC, DIN], mybir.dt.float32, kind="ExternalInput")
    we_d = nc.dram_tensor("w_enc", [DIN, DSAE], mybir.dt.float32, kind="ExternalInput")
    be_d = nc.dram_tensor("b_enc", [1, DSAE], mybir.dt.float32, kind="ExternalInput")
    wd_d = nc.dram_tensor("w_dec", [DSAE, DIN], mybir.dt.bfloat16, kind="ExternalInput")
    bd_d = nc.dram_tensor("b_dec", [1, DIN], mybir.dt.float32, kind="ExternalInput")
    out_d = nc.dram_tensor("xhat", [TPC, DIN], mybir.dt.float32, kind="ExternalOutput")

    with TileContext(nc) as tc:
        with tc.tile_pool(name="cst", bufs=1) as cst, \
             tc.tile_pool(name="big", bufs=1) as big, \
             tc.tile_pool(name="st", bufs=2) as st, \
             tc.tile_pool(name="wenc", bufs=4) as wenc_p, \
             tc.tile_pool(name="wdec", bufs=3) as wdec_p, \
             tc.tile_pool(name="ps", bufs=8, space="PSUM") as psp:

            ident = cst.tile([128, 128], mybir.dt.float32, tag="ident")
            make_identity(nc, ident)
            be_sb = bd_bc = ones1 = None
            if with_benc:
                be_sb = cst.tile([1, DSAE], mybir.dt.float32, tag="be")
                nc.sync.dma_start(be_sb, be_d[:, :])
                ones1 = cst.tile([1, 128], mybir.dt.float32, tag="ones")
                nc.vector.memset(ones1, 1.0)
            if with_bdec:
                bd_row = cst.tile([1, DIN], mybir.dt.float32, tag="bdr")
                nc.sync.dma_start(bd_row, bd_d[:, :])
                bd_bc = cst.tile([128, DIN], mybir.dt.float32, tag="bdb")
                nc.gpsimd.partition_broadcast(bd_bc, bd_row)

            pre = big.tile([128, DSAE], mybir.dt.float32, tag="pre")
            # E^T for a pair of token tiles: column = k*256 + mm*128 + tok
            eT = big.tile([128, KTD * 256], mybir.dt.bfloat16, tag="eT")
            eT3 = eT.rearrange("p (k t) -> p k t", t=256)

            for pair in range(MT // 2):
                for mm in range(2):
                    m = pair * 2 + mm
                    # ---- load + de-bias + transpose x tile ----
                    xm = st.tile([128, DIN], mybir.dt.float32, tag="xm", bufs=1)
                    nc.sync.dma_start(xm, x_d[m * 128:(m + 1) * 128, :])
                    if with_bdec:
                        nc.vector.tensor_sub(xm, xm, bd_bc)
                    xT = st.tile([128, DIN], mybir.dt.float32, tag="xT")
                    for g in range(DIN // 512):
                        ps = psp.tile([128, 512], mybir.dt.float32, tag="ps")
                        for j in range(4):
                            kk = g * 4 + j
                            nc.tensor.transpose(
                                ps[:, j * 128:(j + 1) * 128],
                                xm[:, kk * 128:(kk + 1) * 128], ident)
                        nc.vector.tensor_copy(xT[:, g * 512:(g + 1) * 512], ps)

                    # ---- encode + candidate extraction ----
                    cands = st.tile([128, NCAND], mybir.dt.float32, tag="cands")
                    for c in range(NCH):
                        ps = psp.tile([128, CH], mybir.dt.float32, tag="ps")
                        for k in range(KTE):
                            wt = wenc_p.tile([128, CH], mybir.dt.float32, tag="we")
                            nc.sync.dma_start(
                                wt, we_d[k * 128:(k + 1) * 128, c * CH:(c + 1) * CH])
                            last = (k == KTE - 1) and not with_benc
                            nc.tensor.matmul(ps, xT[:, k * 128:(k + 1) * 128], wt,
                                             start=(k == 0), stop=last)
                        if with_benc:
                            nc.tensor.matmul(ps, ones1, be_sb[:, c * CH:(c + 1) * CH],
                                             start=False, stop=True)
                        nc.vector.tensor_copy(pre[:, c * CH:(c + 1) * CH], ps)
                        for r in range(R_EXT):
                            m8 = cands[:, (c * R_EXT + r) * 8:(c * R_EXT + r + 1) * 8]
                            nc.vector.max(out=m8, in_=ps)
                            nc.vector.match_replace(out=ps, in_to_replace=m8,
                                                    in_values=ps, imm_value=NEG)

                    # ---- exact top-100 of candidates -> threshold ----
                    s8 = st.tile([128, 8], mybir.dt.float32, tag="s8")
                    nrounds = (TOPK + 7) // 8
                    for r in range(nrounds):
                        nc.vector.max(out=s8, in_=cands)
                        if r < nrounds - 1:
                            nc.vector.match_replace(out=cands, in_to_replace=s8,
                                                    in_values=cands, imm_value=NEG)
                    t_col = st.tile([128, 1], mybir.dt.float32, tag="tcol")
                    # rank-100 value; clamp to >0 so the mask implies relu
                    nc.vector.tensor_scalar_max(
                        t_col, s8[:, (TOPK - 1) % 8:(TOPK - 1) % 8 + 1], 1e-30)

                    # ---- t -> broadcast row replicated over partitions ----
                    ps = psp.tile([128, 512], mybir.dt.float32, tag="ps")
                    nc.tensor.transpose(ps[0:1, 0:128], t_col, ident)
                    t_row = st.tile([1, 128], mybir.dt.float32, tag="trow")
                    nc.vector.tensor_copy(t_row, ps[0:1, 0:128])
                    t_rep = st.tile([128, 512], mybir.dt.float32, tag="trep", bufs=1)
                    nc.gpsimd.partition_broadcast(t_rep[:, 0:128], t_row)
                    for rr in range(1, 4):
                        nc.vector.tensor_copy(t_rep[:, rr * 128:(rr + 1) * 128],
                                              t_rep[:, 0:128])

                    # ---- transpose pre, mask, cast -> E^T (bf16) ----
                    ge = st.tile([128, 512], mybir.dt.float32, tag="ge", bufs=1)
                    for g in range(KTD // 4):
                        ps = psp.tile([128, 512], mybir.dt.float32, tag="ps")
                        for j in range(4):
                            kk = g * 4 + j
                            nc.tensor.transpose(ps[:, j * 128:(j + 1) * 128],
                                                pre[:, kk * 128:(kk + 1) * 128], ident)
                        nc.vector.tensor_tensor(out=ge, in0=ps, in1=t_rep,
                                                op=mybir.AluOpType.is_ge)
                        dst = eT3[:, g * 4:(g + 1) * 4, mm * 128:(mm + 1) * 128]
                        src = ps.rearrange("p (j t) -> p j t", j=4)
                        gev = ge.rearrange("p (j t) -> p j t", j=4)
                        nc.vector.tensor_tensor(out=dst, in0=src, in1=gev,
                                                op=mybir.AluOpType.mult)

                # ---- decode the pair: xhat[tok, din] += E^T.T @ W_dec ----
                psd = [[psp.tile([128, 512], mybir.dt.float32, tag="ps",
                                 name=f"psd_{pair}_{mm2}_{c2}")
                        for c2 in range(DIN // 512)] for mm2 in range(2)]
                for k in range(KTD):
                    wd = wdec_p.tile([128, DIN], mybir.dt.bfloat16, tag="wd")
                    nc.sync.dma_start(wd, wd_d[k * 128:(k + 1) * 128, :])
                    for mm in range(2):
                        lhsT = eT[:, k * 256 + mm * 128: k * 256 + (mm + 1) * 128]
                        for c in range(DIN // 512):
                            nc.tensor.matmul(psd[mm][c], lhsT,
                                             wd[:, c * 512:(c + 1) * 512],
                                             start=(k == 0), stop=(k == KTD - 1))
                for mm in range(2):
                    m = pair * 2 + mm
                    xh = st.tile([128, DIN], mybir.dt.float32, tag="xh")
                    for c in range(DIN // 512):
                        if with_bdec:
                            nc.vector.tensor_add(xh[:, c * 512:(c + 1) * 512],
                                                 psd[mm][c], bd_bc[:, c * 512:(c + 1) * 512])
                        else:
                            nc.vector.tensor_copy(xh[:, c * 512:(c + 1) * 512], psd[mm][c])
                    nc.gpsimd.dma_start(out_d[m * 128:(m + 1) * 128, :], xh)

    nc.compile()
    _cache[key] = nc
    return nc


def kernel(x, W_enc, b_enc, W_dec, b_dec):
    x = np.ascontiguousarray(np.asarray(x, dtype=np.float32))
    W_enc = np.ascontiguousarray(np.asarray(W_enc, dtype=np.float32))
    b_enc = np.asarray(b_enc, dtype=np.float32).reshape(1, DSAE)
    W_dec_bf = np.asarray(W_dec, dtype=np.float32).astype(ml_dtypes.bfloat16)
    b_dec = np.asarray(b_dec, dtype=np.float32).reshape(1, DIN)

    nc = _build(bool(np.any(b_enc)), bool(np.any(b_dec)))
    in_maps = []
    for c in range(NCORES):
        in_maps.append({
            "x": x[c * TPC:(c + 1) * TPC],
            "w_enc": W_enc,
            "b_enc": b_enc,
            "w_dec": W_dec_bf,
            "b_dec": b_dec,
        })
    import os
    trace = bool(int(os.environ.get("KERNEL_TRACE", "0")))
    res = run_bass_kernel_spmd(nc, in_maps, core_ids=list(range(NCORES)), trace=trace)
    kernel.last_results = res
    out = np.concatenate([r["xhat"] for r in res.results], axis=0)
    return out.astype(np.float32)


# revision 7
# speedup vs baseline: 301.9284x; 301.9284x over previous
"""TopK sparse autoencoder (encode -> per-token top-100 mask -> decode) on 8 TRN2 cores.

Sharding: data-parallel over the 4096-token batch (512 tokens/core), weights
replicated. Per core:
  pre  = (x - b_dec) @ W_enc + b_enc          (fp32 matmul on PE, exact selection)
  t    = 100th largest of relu(pre) per token (DVE max8/match_replace:
         top-24 of each 512-wide chunk extracted destructively from PSUM
         during evacuation, then exact top-100 of the 768 candidates)
  E    = pre * (pre >= t)                     (masked in transposed layout)
  xhat = E @ W_dec + b_dec                    (bf16 matmul, E^T tiles stationary)

The top-24-per-chunk candidate set provably contains the global top-100 as
long as no 512-chunk holds more than 24 of a row's top-100 (true with huge
margin for iid inputs; max observed is 15).
"""
import numpy as np
import ml_dtypes

import concourse.bacc as bacc
import concourse.mybir as mybir
from concourse.tile import TileContext
from concourse.masks import make_identity
from concourse.bass_utils import run_bass_kernel_spmd

B, DIN, DSAE, TOPK = 4096, 2048, 16384, 100
NCORES = 8
TPC = B // NCORES            # 512 tokens per core
MT = TPC // 128              # 4 token tiles per core
CH = 512                     # encode chunk width == one PSUM bank (fp32)
NCH = DSAE // CH             # 32 chunks
KTE = DIN // 128             # 16 contraction slices for encode
KTD = DSAE // 128            # 128 contraction slices for decode
R_EXT = 3                    # extraction rounds per chunk -> top-24 candidates
NCAND = NCH * R_EXT * 8      # 768 candidates per token
NEG = -1e30

_cache = {}


def _build(with_benc: bool, with_bdec: bool, mode: str = "bf3"):
    key = (with_benc, with_bdec, mode)
    if key in _cache:
        return _cache[key]

    nc = bacc.Bacc()
    x_d = nc.dram_tensor("x", [TPC, DIN], mybir.dt.float32, kind="ExternalInput")
    if mode == "f32":
        we_d = nc.dram_tensor("w_enc", [DIN, DSAE], mybir.dt.float32, kind="ExternalInput")
    else:
        weh_d = nc.dram_tensor("w_enc_h", [DIN, DSAE], mybir.dt.bfloat16, kind="ExternalInput")
        wel_d = nc.dram_tensor("w_enc_l", [DIN, DSAE], mybir.dt.bfloat16, kind="ExternalInput")
    be_d = nc.dram_tensor("b_enc", [1, DSAE], mybir.dt.float32, kind="ExternalInput")
    wd_d = nc.dram_tensor("w_dec", [DSAE, DIN], mybir.dt.bfloat16, kind="ExternalInput")
    bd_d = nc.dram_tensor("b_dec", [1, DIN], mybir.dt.float32, kind="ExternalInput")
    out_d = nc.dram_tensor("xhat", [TPC, DIN], mybir.dt.float32, kind="ExternalOutput")

    with TileContext(nc) as tc:
        with tc.tile_pool(name="cst", bufs=1) as cst, \
             tc.tile_pool(name="big", bufs=1) as big, \
             tc.tile_pool(name="st", bufs=2) as st, \
             tc.tile_pool(name="wenc", bufs=4) as wenc_p, \
             tc.tile_pool(name="wdec", bufs=3) as wdec_p, \
             tc.tile_pool(name="ps", bufs=8, space="PSUM") as psp:

            ident = cst.tile([128, 128], mybir.dt.float32, tag="ident")
            make_identity(nc, ident)
            be_sb = bd_bc = ones1 = None
            if with_benc:
                be_sb = cst.tile([1, DSAE], mybir.dt.float32, tag="be")
                nc.sync.dma_start(be_sb, be_d[:, :])
                ones1 = cst.tile([1, 128], mybir.dt.float32, tag="ones")
                nc.vector.memset(ones1, 1.0)
            if with_bdec:
                bd_row = cst.tile([1, DIN], mybir.dt.float32, tag="bdr")
                nc.sync.dma_start(bd_row, bd_d[:, :])
                bd_bc = cst.tile([128, DIN], mybir.dt.float32, tag="bdb")
                nc.gpsimd.partition_broadcast(bd_bc, bd_row)

            pre = big.tile([128, DSAE], mybir.dt.float32, tag="pre")
            # E^T for a pair of token tiles: column = k*256 + mm*128 + tok
            eT = big.tile([128, KTD * 256], mybir.dt.bfloat16, tag="eT")
            eT3 = eT.rearrange("p (k t) -> p k t", t=256)

            for pair in range(MT // 2):
                for mm in range(2):
                    m = pair * 2 + mm
                    # ---- load + de-bias + transpose x tile ----
                    xm = st.tile([128, DIN], mybir.dt.float32, tag="xm", bufs=1)
                    nc.sync.dma_start(xm, x_d[m * 128:(m + 1) * 128, :])
                    if with_bdec:
                        nc.vector.tensor_sub(xm, xm, bd_bc)
                    if mode == "f32":
                        xT = st.tile([128, DIN], mybir.dt.float32, tag="xT", bufs=1)
                    else:
                        xTh = st.tile([128, DIN], mybir.dt.bfloat16, tag="xTh", bufs=1)
                        xTl = st.tile([128, DIN], mybir.dt.bfloat16, tag="xTl", bufs=1)
                    for g in range(DIN // 512):
                        ps = psp.tile([128, 512], mybir.dt.float32, tag="ps")
                        for j in range(4):
                            kk = g * 4 + j
                            nc.tensor.transpose(
                                ps[:, j * 128:(j + 1) * 128],
                                xm[:, kk * 128:(kk + 1) * 128], ident)
                        sl = slice(g * 512, (g + 1) * 512)
                        if mode == "f32":
                            nc.vector.tensor_copy(xT[:, sl], ps)
                        else:
                            nc.vector.tensor_copy(xTh[:, sl], ps)
                            nc.vector.tensor_sub(xTl[:, sl], ps, xTh[:, sl])

                    # ---- encode + candidate extraction ----
                    cands = st.tile([128, NCAND], mybir.dt.float32, tag="cands")
                    KG = 4  # k-slices fetched per W DMA
                    for c in range(NCH):
                        ps = psp.tile([128, CH], mybir.dt.float32, tag="ps")
                        csl = slice(c * CH, (c + 1) * CH)
                        for kg in range(KTE // KG):
                            rows = slice(kg * KG * 128, (kg + 1) * KG * 128)
                            if mode == "f32":
                                wt = wenc_p.tile([128, KG * CH], mybir.dt.float32,
                                                 tag="we", name=f"wt_{m}_{c}_{kg}")
                                nc.sync.dma_start(
                                    wt.rearrange("p (k n) -> p k n", k=KG),
                                    we_d[rows, csl].rearrange("(k p) n -> p k n", p=128))
                            else:
                                wh = wenc_p.tile([128, KG * CH], mybir.dt.bfloat16,
                                                 tag="we", name=f"wh_{m}_{c}_{kg}")
                                wl = wenc_p.tile([128, KG * CH], mybir.dt.bfloat16,
                                                 tag="we", name=f"wl_{m}_{c}_{kg}")
                                nc.sync.dma_start(
                                    wh.rearrange("p (k n) -> p k n", k=KG),
                                    weh_d[rows, csl].rearrange("(k p) n -> p k n", p=128))
                                nc.sync.dma_start(
                                    wl.rearrange("p (k n) -> p k n", k=KG),
                                    wel_d[rows, csl].rearrange("(k p) n -> p k n", p=128))
                            for kk in range(KG):
                                k = kg * KG + kk
                                ksl = slice(k * 128, (k + 1) * 128)
                                wsl = slice(kk * CH, (kk + 1) * CH)
                                last = (k == KTE - 1) and not with_benc
                                if mode == "f32":
                                    nc.tensor.matmul(ps, xT[:, ksl], wt[:, wsl],
                                                     start=(k == 0), stop=last)
                                else:
                                    nc.tensor.matmul(ps, xTh[:, ksl], wh[:, wsl],
                                                     start=(k == 0), stop=False)
                                    nc.tensor.matmul(ps, xTh[:, ksl], wl[:, wsl],
                                                     start=False, stop=False)
                                    nc.tensor.matmul(ps, xTl[:, ksl], wh[:, wsl],
                                                     start=False, stop=last)
                        if with_benc:
                            nc.tensor.matmul(ps, ones1, be_sb[:, c * CH:(c + 1) * CH],
                                             start=False, stop=True)
                        nc.vector.tensor_copy(pre[:, c * CH:(c + 1) * CH], ps)
                        for r in range(R_EXT):
                            m8 = cands[:, (c * R_EXT + r) * 8:(c * R_EXT + r + 1) * 8]
                            nc.vector.max(out=m8, in_=ps)
                            nc.vector.match_replace(out=ps, in_to_replace=m8,
                                                    in_values=ps, imm_value=NEG)

                    # ---- exact top-100 of candidates -> threshold ----
                    s8 = st.tile([128, 8], mybir.dt.float32, tag="s8")
                    nrounds = (TOPK + 7) // 8
                    for r in range(nrounds):
                        nc.vector.max(out=s8, in_=cands)
                        if r < nrounds - 1:
                            nc.vector.match_replace(out=cands, in_to_replace=s8,
                                                    in_values=cands, imm_value=NEG)
                    t_col = st.tile([128, 1], mybir.dt.float32, tag="tcol")
                    # rank-100 value; clamp to >0 so the mask implies relu
                    nc.vector.tensor_scalar_max(
                        t_col, s8[:, (TOPK - 1) % 8:(TOPK - 1) % 8 + 1], 1e-30)

                    # ---- t -> broadcast row replicated over partitions ----
                    ps = psp.tile([128, 512], mybir.dt.float32, tag="ps")
                    nc.tensor.transpose(ps[0:1, 0:128], t_col, ident)
                    t_row = st.tile([1, 128], mybir.dt.float32, tag="trow")
                    nc.vector.tensor_copy(t_row, ps[0:1, 0:128])
                    t_rep = st.tile([128, 512], mybir.dt.float32, tag="trep", bufs=1)
                    nc.gpsimd.partition_broadcast(t_rep[:, 0:128], t_row)
                    for rr in range(1, 4):
                        nc.vector.tensor_copy(t_rep[:, rr * 128:(rr + 1) * 128],
                                              t_rep[:, 0:128])

                    # ---- transpose pre, mask, cast -> E^T (bf16) ----
                    ge = st.tile([128, 512], mybir.dt.float32, tag="ge", bufs=1)
                    for g in range(KTD // 4):
                        ps = psp.tile([128, 512], mybir.dt.float32, tag="ps")
                        for j in range(4):
                            kk = g * 4 + j
                            nc.tensor.transpose(ps[:, j * 128:(j + 1) * 128],
                                                pre[:, kk * 128:(kk + 1) * 128], ident)
                        nc.vector.tensor_tensor(out=ge, in0=ps, in1=t_rep,
                                                op=mybir.AluOpType.is_ge)
                        dst = eT3[:, g * 4:(g + 1) * 4, mm * 128:(mm + 1) * 128]
                        src = ps.rearrange("p (j t) -> p j t", j=4)
                        gev = ge.rearrange("p (j t) -> p j t", j=4)
                        nc.vector.tensor_tensor(out=dst, in0=src, in1=gev,
                                                op=mybir.AluOpType.mult)

                # ---- decode the pair: xhat[tok, din] += E^T.T @ W_dec ----
                psd = [[psp.tile([128, 512], mybir.dt.float32, tag="ps",
                                 name=f"psd_{pair}_{mm2}_{c2}")
                        for c2 in range(DIN // 512)] for mm2 in range(2)]
                for k in range(KTD):
                    wd = wdec_p.tile([128, DIN], mybir.dt.bfloat16, tag="wd")
                    nc.sync.dma_start(wd, wd_d[k * 128:(k + 1) * 128, :])
                    for mm in range(2):
                        lhsT = eT[:, k * 256 + mm * 128: k * 256 + (mm + 1) * 128]
                        for c in range(DIN // 512):
                            nc.tensor.matmul(psd[mm][c], lhsT,
                                             wd[:, c * 512:(c + 1) * 512],
                                             start=(k == 0), stop=(k == KTD - 1))
                for mm in range(2):
                    m = pair * 2 + mm
                    xh = st.tile([128, DIN], mybir.dt.float32, tag="xh", bufs=1)
                    for c in range(DIN // 512):
                        if with_bdec:
                            nc.vector.tensor_add(xh[:, c * 512:(c + 1) * 512],
                                                 psd[mm][c], bd_bc[:, c * 512:(c + 1) * 512])
                        else:
                            nc.vector.tensor_copy(xh[:, c * 512:(c + 1) * 512], psd[mm][c])
                    nc.gpsimd.dma_start(out_d[m * 128:(m + 1) * 128, :], xh)

    nc.compile()
    _cache[key] = nc
    return nc


def kernel(x, W_enc, b_enc, W_dec, b_dec):
    x = np.ascontiguousarray(np.asarray(x, dtype=np.float32))
    W_enc = np.ascontiguousarray(np.asarray(W_enc, dtype=np.float32))
    b_enc = np.asarray(b_enc, dtype=np.float32).reshape(1, DSAE)
    W_dec_bf = np.asarray(W_dec, dtype=np.float32).astype(ml_dtypes.bfloat16)
    b_dec = np.asarray(b_dec, dtype=np.float32).reshape(1, DIN)

    import os
    mode = os.environ.get("KERNEL_MODE", "bf3")
    nc = _build(bool(np.any(b_enc)), bool(np.any(b_dec)), mode)
    in_maps = []
    for c in range(NCORES):
        m = {
            "x": x[c * TPC:(c + 1) * TPC],
            "b_enc": b_enc,
            "w_dec": W_dec_bf,
            "b_dec": b_dec,
        }
        if mode == "f32":
            m["w_enc"] = W_enc
        else:
            if c == 0:
                Wh = W_enc.astype(ml_dtypes.bfloat16)
                Wl = (W_enc - Wh.astype(np.float32)).astype(ml_dtypes.bfloat16)
            m["w_enc_h"] = Wh
            m["w_enc_l"] = Wl
        in_maps.append(m)
    import os
    trace = bool(int(os.environ.get("KERNEL_TRACE", "0")))
    res = run_bass_kernel_spmd(nc, in_maps, core_ids=list(range(NCORES)), trace=trace)
    kernel.last_results = res
    out = np.concatenate([r["xhat"] for r in res.results], axis=0)
    return out.astype(np.float32)
